# revision 31
# baseline (speedup 1.0000x reference)
"""RNN-T Joiner kernel for Trainium2, data-parallel over batch N across 8 NeuronCores.

Per core (one batch element):
  enc_T[J,T] = enc_W @ x_enc.T          (projection, bf16 matmul, fp32 accum)
  dec_T[J,U] = dec_W @ x_dec.T + (enc_b+dec_b)
  act[J,(u,t)] = tanh(enc_T[:,t] + dec_T[:,u])   (ScalarE, add fused as per-partition bias)
  out[u,t,:] = act.T @ out_W.T + out_b           (PE, act stationary, fp32 PSUM)

Output is written to DRAM in bf16 with a DMA-friendly layout [U, P, TB, V]
(per u: one contiguous 512 KB block, 4000 B per-partition lines); the host
unscrambles to [T, U, V] and upcasts to fp32 during the gather.  bf16 output
rounding costs ~4e-3 relative error (gate is 2e-2); it halves store traffic
and doubles the DMA line length vs the fp32 [T,U,V] layout.

Inputs are staged host-side in partition-major [P, C, free] layouts so input
DMAs move 2-4 KB contiguous per-partition lines instead of 1 KB, split across
the sync and gpsimd queues to shorten the prologue.

All layout transposes (x.T, W.T) are done host-side as part of sharding, so the
device program has zero on-chip transposes.
"""

import sys

import numpy as np

try:
    import concourse.bass as bass
except ImportError:
    sys.path.insert(0, "/opt/trn_rl_repo")
    import concourse.bass as bass

import ml_dtypes

import concourse.mybir as mybir
import concourse.tile as tile
from concourse import bacc
from concourse.bass import ds, ts
from concourse.bass_utils import run_bass_kernel_spmd

N, T, U = 8, 512, 64
E = D = J = 512
V = 500
P = 128
JC = J // P  # 4 chunks of J on partitions
TB = T // P  # 4 blocks of T rows per output tile
EC = E // P  # 4 chunks of E (contraction) on partitions
F32 = mybir.dt.float32
BF16 = mybir.dt.bfloat16

NUM_CORES = 8

# PE warm-up dummy matmul counts (each ~53 ns cold), split around the out_b
# broadcast.  Sized so warm-up ends as the WeT/xT half1 DMA semaphores fire.
WARM_A = 98


def build_nc() -> bass.Bass:
    nc = bacc.Bacc(
        "TRN2", target_bir_lowering=False, debug=False, num_devices=NUM_CORES
    )
    # all inputs pre-arranged host-side as [P, chunk, free]
    # projection operands arrive as TWO contiguous ek-half tensors each, so
    # the half1 DMAs move 2-4 KB per-partition lines at full rate and the
    # first enc matmuls gate on 256 KB instead of 512 KB
    xTa = nc.declare_dram_parameter("xTa", [P, 2, T], BF16, isOutput=False)
    xTb = nc.declare_dram_parameter("xTb", [P, 2, T], BF16, isOutput=False)
    dT = nc.declare_dram_parameter("dT", [P, EC, U], BF16, isOutput=False)
    WeTa = nc.declare_dram_parameter("WeTa", [P, 2, J], BF16, isOutput=False)
    WeTb = nc.declare_dram_parameter("WeTb", [P, 2, J], BF16, isOutput=False)
    WdTa = nc.declare_dram_parameter("WdTa", [P, 2, J], BF16, isOutput=False)
    WdTb = nc.declare_dram_parameter("WdTb", [P, 2, J], BF16, isOutput=False)
    WoT = nc.declare_dram_parameter("WoT", [P, JC, V], BF16, isOutput=False)
    cb = nc.declare_dram_parameter("cb", [P, JC], F32, isOutput=False)
    # out_b replicated host-side to [P, V] (128 KB): cheaper than an
    # on-device broadcast, whose 1 KB DMA semaphore fires no earlier than
    # ~11.9 us and forced a PE idle bubble mid-warmup
    ob = nc.declare_dram_parameter("ob", [P, V], BF16, isOutput=False)
    # [u, p, tb, v]: per u one contiguous 512 KB block, per partition 4000 B
    out = nc.declare_dram_parameter("out", [U, P, TB, V], BF16, isOutput=True)

    with tile.TileContext(nc) as tc:
        with (
            tc.tile_pool(name="const", bufs=1) as const_pool,
            tc.tile_pool(name="acts", bufs=3) as act_pool,
            tc.tile_pool(name="otile", bufs=3) as out_pool,
            tc.tile_pool(name="psum", bufs=8, space="PSUM") as psum_pool,
        ):
            # ---- persistent SBUF tensors -------------------------------------
            # chunked layouts: [P, chunk, free]
            xT_sb = const_pool.tile([P, EC, T], BF16, tag="xT")
            dT_sb = const_pool.tile([P, EC, U], BF16, tag="dT")
            WeT_sb = const_pool.tile([P, EC, J], BF16, tag="WeT")
            WdT_sb = const_pool.tile([P, EC, J], BF16, tag="WdT")
            Wo_sb = const_pool.tile([P, JC, V], BF16, tag="WoT")
            cb_sb = const_pool.tile([P, JC], F32, tag="cb")
            ob_sb = const_pool.tile([P, V], BF16, tag="ob")
            enc_sb = const_pool.tile([P, JC, T], F32, tag="encT")
            dec_sb = const_pool.tile([P, JC, U], F32, tag="decT")

            # warmup operand memsets go FIRST on gpsimd (its engine wakes
            # earliest, ~6.1 us) so the PE's dummy matmuls can start ~6.9 us,
            # before the DMA trigger instructions have even issued.
            warm_sb = const_pool.tile([P, 64], BF16, tag="warm")
            warm_act = const_pool.tile([P, 64], BF16, tag="warm_act")
            nc.gpsimd.memset(warm_sb[:], 0.0)

            # input DMAs: spread across FOUR trigger queues (sync, vector,
            # scalar, gpsimd) so the transfers all start ~7 us, and split the
            # projection operands into ek halves so the first enc matmuls only
            # wait on half1 (256 KB) instead of the full 512 KB tensor.  The
            # ek-outer projection order below consumes exactly in this order.
            nc.scalar.dma_start(WeT_sb[:, :2, :], WeTa[:])
            nc.gpsimd.dma_start(xT_sb[:, :2, :], xTa[:])
            nc.scalar.dma_start(WeT_sb[:, 2:, :], WeTb[:])
            nc.gpsimd.dma_start(xT_sb[:, 2:, :], xTb[:])
            nc.scalar.dma_start(WdT_sb[:, :2, :], WdTa[:])
            nc.scalar.dma_start(WdT_sb[:, 2:, :], WdTb[:])
            nc.scalar.dma_start(dT_sb[:], dT[:])
            nc.sync.dma_start(cb_sb[:], cb[:])
            nc.sync.dma_start(ob_sb[:], ob[:])
            nc.gpsimd.dma_start(Wo_sb[:, :2, :], WoT[:, :2, :])
            nc.gpsimd.dma_start(Wo_sb[:, 2:, :], WoT[:, 2:, :])

            # ---- projections -------------------------------------------------
            # enc_T[J,T]: lhsT = WeT chunk [E_k, J_m], rhs = xT chunk [E_k, T]
            # ek outer so the first matmuls only need chunks 0-1 of the DMAs;
            # enc and dec interleaved per chunk so each DMA round feeds both.
            ps_enc = [
                psum_pool.tile([P, T], F32, tag="ps", name=f"ps_enc_{jm}")
                for jm in range(JC)
            ]
            ps_dec = [
                psum_pool.tile([P, T], F32, tag="ps", name=f"ps_dec_{jm}")
                for jm in range(JC)
            ]

            # PE warm-up: dummy matmuls while the input DMAs land, so the HAM
            # clock-gate lifts (1.2 -> 2.4 GHz) before the projections issue.
            # Target ps_dec[3]'s bank (the last projection group to issue) so
            # the WAW ordering with the warm-up delays the pipeline least.
            # The group's start=True resets the bank afterwards.  The count is
            # sized so the last dummy ends right as the WeT/xT half1 DMA
            # semaphores fire - any longer delays the projections, any shorter
            # leaves a PE idle gap that re-throttles the clock to 1.2 GHz.
            # dummy tanh while the input DMAs land: hoists ScalarE's 1.3 us
            # ACT_TABLE_LOAD off the dec_sb -> first-tanh critical path
            nc.scalar.activation(
                warm_act[:], warm_sb[:], mybir.ActivationFunctionType.Tanh
            )
            for w in range(WARM_A):
                nc.tensor.matmul(
                    ps_dec[3][:64, :64],
                    lhsT=warm_sb[:, :64],
                    rhs=warm_sb[:],
                    start=True,
                    stop=True,
                    skip_group_check=True,
                )

            # ek-outer: an ek pass touches only one 128-row chunk of WeT/xT,
            # so the first 8 enc matmuls gate only on the half1 DMAs (256 KB)
            # instead of the whole 512 KB tensors.  All four enc (and dec)
            # PSUM groups accumulate in parallel and close on the ek=3 pass.
            def enc_pass(ek):
                for jm in range(JC):
                    nc.tensor.matmul(
                        ps_enc[jm][:],
                        lhsT=WeT_sb[:, ek, ts(jm, P)],
                        rhs=xT_sb[:, ek, :],
                        start=(ek == 0),
                        stop=(ek == EC - 1),
                        skip_group_check=True,
                    )

            def dec_pass(ek):
                for jm in range(JC):
                    nc.tensor.matmul(
                        ps_dec[jm][:, :U],
                        lhsT=WdT_sb[:, ek, ts(jm, P)],
                        rhs=dT_sb[:, ek, :],
                        start=(ek == 0),
                        stop=(ek == EC - 1),
                        skip_group_check=True,
                    )

            def enc_mm(jm, ek, stop=False):
                nc.tensor.matmul(
                    ps_enc[jm][:],
                    lhsT=WeT_sb[:, ek, ts(jm, P)],
                    rhs=xT_sb[:, ek, :],
                    start=False,
                    stop=stop,
                    skip_group_check=True,
                )

            enc_pass(0)
            enc_pass(1)
            # jm=0 closes FIRST so its DVE evac (halved: tanh needs only
            # T-half0 to start) and the scalar chain behind it overlap the
            # rest of the projections.  dec sits after the jm1-3 ek=2 filler
            # because its WdT/dT semaphores are the last to fire; its ScalarE
            # bias-add evacuations then drain during the final enc matmuls.
            enc_mm(0, 2)
            enc_mm(0, 3, stop=True)
            nc.vector.tensor_copy(enc_sb[:, 0, : T // 2], ps_enc[0][:, : T // 2])
            nc.vector.tensor_copy(enc_sb[:, 0, T // 2 :], ps_enc[0][:, T // 2 :])
            for jm in (1, 2, 3):
                enc_mm(jm, 2)
            dec_pass(0)
            dec_pass(1)
            dec_pass(2)
            dec_pass(3)

            def dec_evac(jm):
                nc.scalar.activation(
                    dec_sb[:, jm, :],
                    ps_dec[jm][:, :U],
                    mybir.ActivationFunctionType.Identity,
                    bias=cb_sb[:, jm : jm + 1],
                )

            # only jm=0's bias-add is issued here; jm1-3 are interleaved into
            # the u=0 tanh loop so they don't sit ahead of tanh(u0, jc0) on
            # the in-order ScalarE queue (they'd delay the first u matmul)
            dec_evac(0)
            for jm in (1, 2, 3):
                enc_mm(jm, 3, stop=True)
                nc.vector.tensor_copy(enc_sb[:, jm, :], ps_enc[jm][:])

            # ---- main loop over u -------------------------------------------
            for u in range(U):
                act_t = act_pool.tile([P, JC, T], BF16, tag="act", name=f"act_{u}")
                for jc in range(JC):
                    if u == 0 and jc == 0:
                        for h in range(2):
                            nc.scalar.activation(
                                act_t[:, 0, ts(h, T // 2)],
                                enc_sb[:, 0, ts(h, T // 2)],
                                mybir.ActivationFunctionType.Tanh,
                                bias=dec_sb[:, 0, u : u + 1],
                            )
                    else:
                        if u == 0:
                            dec_evac(jc)
                        nc.scalar.activation(
                            act_t[:, jc, :],
                            enc_sb[:, jc, :],
                            mybir.ActivationFunctionType.Tanh,
                            bias=dec_sb[:, jc, u : u + 1],
                        )
                ot = out_pool.tile([P, TB, V], BF16, tag="ot", name=f"ot_{u}")
                # First two u: jc-outer over the four tb accumulation groups,
                # so the first four matmuls need only tanh chunk 0 and the PE
                # never stalls on a chunk ScalarE hasn't produced yet.
                # Later u (act always ready ahead): tb-outer, so each group
                # closes early and its evacuation overlaps the remaining
                # matmuls - keeps the final-u evacs off the tail.
                pss = [
                    psum_pool.tile([P, T], F32, tag="ps", name=f"ps_{u}_{tb}")
                    for tb in range(TB)
                ]
                order = (
                    [(jc, tb) for jc in range(JC) for tb in range(TB)]
                    if u < 2
                    else [(jc, tb) for tb in range(TB) for jc in range(JC)]
                )
                for jc, tb in order:
                    nc.tensor.matmul(
                        pss[tb][:, :V],
                        lhsT=act_t[:, jc, ts(tb, P)],
                        rhs=Wo_sb[:, jc, :],
                        start=(jc == 0),
                        stop=(jc == JC - 1),
                        skip_group_check=True,
                    )
                for tb in range(TB):
                    ps = pss[tb]
                    nc.vector.tensor_add(ot[:, tb, :], ps[:, :V], ob_sb[:])
                    if u == U - 1:
                        # final u: early tb-pair store, then single-tb stores
                        # so the last transfer (the tail's critical path) is
                        # only 128 KB and departs as soon as its evac lands.
                        if tb == 1:
                            nc.sync.dma_start(out[u, :, :2, :], ot[:, :2, :])
                        elif tb == 2:
                            nc.sync.dma_start(out[u, :, 2:3, :], ot[:, 2:3, :])
                        elif tb == TB - 1:
                            nc.gpsimd.dma_start(out[u, :, 3:, :], ot[:, 3:, :])
                if u < U - 1:
                    # one contiguous 512 KB store per u (4000 B per partition);
                    # per-tb slices of this layout have only 1000 B lines, so
                    # splitting penultimate stores measurably loses bandwidth
                    nc.sync.dma_start(out[u], ot[:])

    nc.compile()
    return nc


_CACHED_NC = None


def _get_nc():
    global _CACHED_NC
    if _CACHED_NC is None:
        _CACHED_NC = build_nc()
    return _CACHED_NC


def make_in_maps(
    encoder_out, decoder_out, enc_W, enc_b, dec_W, dec_b, out_W, out_b
) -> list[dict]:
    bf = ml_dtypes.bfloat16
    f32 = np.float32

    def t_pc(a):  # [F, K] -> K on partitions chunk-major: [P, K//P, F], bf16
        a = np.asarray(a, dtype=f32).T  # [K, F]
        k, f = a.shape
        return np.ascontiguousarray(
            a.reshape(k // P, P, f).transpose(1, 0, 2)
        ).astype(bf)

    WeT = t_pc(enc_W)  # [P, EC, J]
    WdT = t_pc(dec_W)  # [P, EC, J]
    WoT = t_pc(out_W)  # [P, JC, V]
    cb = (
        (np.asarray(enc_b, f32) + np.asarray(dec_b, f32))
        .reshape(JC, P)
        .T.copy()
    )  # [P, JC]
    ob = np.broadcast_to(
        np.asarray(out_b, f32).astype(bf).reshape(1, V), (P, V)
    ).copy()  # [P, V] replicated

    encoder_out = np.asarray(encoder_out, f32)
    decoder_out = np.asarray(decoder_out, f32)

    def halves(a):  # [P, EC, F] -> two contiguous [P, 2, F]
        return (
            np.ascontiguousarray(a[:, :2, :]),
            np.ascontiguousarray(a[:, 2:, :]),
        )

    WeTa, WeTb = halves(WeT)
    WdTa, WdTb = halves(WdT)
    in_maps = []
    for i in range(NUM_CORES):
        xTa, xTb = halves(t_pc(encoder_out[i]))
        in_maps.append(
            {
                "xTa": xTa,
                "xTb": xTb,
                "dT": t_pc(decoder_out[i]),  # [P, EC, U]
                "WeTa": WeTa,
                "WeTb": WeTb,
                "WdTa": WdTa,
                "WdTb": WdTb,
                "WoT": WoT,
                "cb": cb,
                "ob": ob,
            }
        )
    return in_maps


def unscramble(dev_out: np.ndarray) -> np.ndarray:
    """[U, P, TB, V] bf16 device layout -> [T, U, V] fp32 (t = tb*P + p)."""
    return (
        np.asarray(dev_out)
        .transpose(2, 1, 0, 3)  # [TB, P, U, V]
        .reshape(T, U, V)
        .astype(np.float32)
    )


def run(inputs: dict, trace: bool = False):
    """Returns (full_output, BassKernelResults)."""
    nc = _get_nc()
    in_maps = make_in_maps(**inputs)
    res = run_bass_kernel_spmd(
        nc, in_maps, core_ids=list(range(NUM_CORES)), trace=trace
    )
    out = np.stack(
        [unscramble(res.results[i]["out"]) for i in range(NUM_CORES)], axis=0
    )  # (N, T, U, V)
    return np.ascontiguousarray(out, dtype=np.float32), res


def kernel(**inputs) -> np.ndarray:
    out, _ = run(inputs, trace=False)
    return out



# revision 32
# speedup vs baseline: 1.0140x; 1.0140x over previous
"""RNN-T Joiner kernel for Trainium2, data-parallel over batch N across 8 NeuronCores.

Per core (one batch element):
  enc_T[J,T] = enc_W @ x_enc.T          (projection, bf16 matmul, fp32 accum)
  dec_T[J,U] = dec_W @ x_dec.T + (enc_b+dec_b)
  act[J,(u,t)] = tanh(enc_T[:,t] + dec_T[:,u])   (ScalarE, add fused as per-partition bias)
  out[u,t,:] = act.T @ out_W.T + out_b           (PE, act stationary, fp32 PSUM)

Output is written to DRAM in bf16 with a DMA-friendly layout [U, P, TB, V]
(per u: one contiguous 512 KB block, 4000 B per-partition lines); the host
unscrambles to [T, U, V] and upcasts to fp32 during the gather.  bf16 output
rounding costs ~4e-3 relative error (gate is 2e-2); it halves store traffic
and doubles the DMA line length vs the fp32 [T,U,V] layout.

Inputs are staged host-side in partition-major [P, C, free] layouts so input
DMAs move 2-4 KB contiguous per-partition lines instead of 1 KB, split across
the sync and gpsimd queues to shorten the prologue.

All layout transposes (x.T, W.T) are done host-side as part of sharding, so the
device program has zero on-chip transposes.
"""

import sys

import numpy as np

try:
    import concourse.bass as bass
except ImportError:
    sys.path.insert(0, "/opt/trn_rl_repo")
    import concourse.bass as bass

import ml_dtypes

import concourse.mybir as mybir
import concourse.tile as tile
from concourse import bacc
from concourse.bass import ds, ts
from concourse.bass_utils import run_bass_kernel_spmd

N, T, U = 8, 512, 64
E = D = J = 512
V = 500
P = 128
JC = J // P  # 4 chunks of J on partitions
TB = T // P  # 4 blocks of T rows per output tile
EC = E // P  # 4 chunks of E (contraction) on partitions
F32 = mybir.dt.float32
BF16 = mybir.dt.bfloat16

NUM_CORES = 8

# PE warm-up dummy matmul counts (each ~53 ns cold), split around the out_b
# broadcast.  Sized so warm-up ends as the WeT/xT half1 DMA semaphores fire.
WARM_A = 98


def build_nc() -> bass.Bass:
    nc = bacc.Bacc(
        "TRN2", target_bir_lowering=False, debug=False, num_devices=NUM_CORES
    )
    # all inputs pre-arranged host-side as [P, chunk, free]
    # projection operands arrive as TWO contiguous ek-half tensors each, so
    # the half1 DMAs move 2-4 KB per-partition lines at full rate and the
    # first enc matmuls gate on 256 KB instead of 512 KB
    xTa = nc.declare_dram_parameter("xTa", [P, 2, T], BF16, isOutput=False)
    xTb = nc.declare_dram_parameter("xTb", [P, 2, T], BF16, isOutput=False)
    dT = nc.declare_dram_parameter("dT", [P, EC, U], BF16, isOutput=False)
    WeTa = nc.declare_dram_parameter("WeTa", [P, 2, J], BF16, isOutput=False)
    WeTb = nc.declare_dram_parameter("WeTb", [P, 2, J], BF16, isOutput=False)
    WdTa = nc.declare_dram_parameter("WdTa", [P, 2, J], BF16, isOutput=False)
    WdTb = nc.declare_dram_parameter("WdTb", [P, 2, J], BF16, isOutput=False)
    WoT = nc.declare_dram_parameter("WoT", [P, JC, V], BF16, isOutput=False)
    cb = nc.declare_dram_parameter("cb", [P, JC], F32, isOutput=False)
    # out_b replicated host-side to [P, V] (128 KB): cheaper than an
    # on-device broadcast, whose 1 KB DMA semaphore fires no earlier than
    # ~11.9 us and forced a PE idle bubble mid-warmup
    ob = nc.declare_dram_parameter("ob", [P, V], BF16, isOutput=False)
    # [u, p, tb, v]: per u one contiguous 512 KB block, per partition 4000 B
    out = nc.declare_dram_parameter("out", [U, P, TB, V], BF16, isOutput=True)

    with tile.TileContext(nc) as tc:
        with (
            tc.tile_pool(name="const", bufs=1) as const_pool,
            tc.tile_pool(name="acts", bufs=3) as act_pool,
            tc.tile_pool(name="otile", bufs=3) as out_pool,
            tc.tile_pool(name="psum", bufs=8, space="PSUM") as psum_pool,
        ):
            # ---- persistent SBUF tensors -------------------------------------
            # chunked layouts: [P, chunk, free]
            xT_sb = const_pool.tile([P, EC, T], BF16, tag="xT")
            dT_sb = const_pool.tile([P, EC, U], BF16, tag="dT")
            WeT_sb = const_pool.tile([P, EC, J], BF16, tag="WeT")
            WdT_sb = const_pool.tile([P, EC, J], BF16, tag="WdT")
            Wo_sb = const_pool.tile([P, JC, V], BF16, tag="WoT")
            cb_sb = const_pool.tile([P, JC], F32, tag="cb")
            ob_sb = const_pool.tile([P, V], BF16, tag="ob")
            enc_sb = const_pool.tile([P, JC, T], F32, tag="encT")
            dec_sb = const_pool.tile([P, JC, U], F32, tag="decT")

            # warmup operand memsets go FIRST on gpsimd (its engine wakes
            # earliest, ~6.1 us) so the PE's dummy matmuls can start ~6.9 us,
            # before the DMA trigger instructions have even issued.
            warm_sb = const_pool.tile([P, 64], BF16, tag="warm")
            warm_act = const_pool.tile([P, 64], BF16, tag="warm_act")
            nc.gpsimd.memset(warm_sb[:], 0.0)

            # input DMAs: spread across FOUR trigger queues (sync, vector,
            # scalar, gpsimd) so the transfers all start ~7 us, and split the
            # projection operands into ek halves so the first enc matmuls only
            # wait on half1 (256 KB) instead of the full 512 KB tensor.  The
            # ek-outer projection order below consumes exactly in this order.
            nc.scalar.dma_start(WeT_sb[:, :2, :], WeTa[:])
            nc.gpsimd.dma_start(xT_sb[:, :2, :], xTa[:])
            nc.scalar.dma_start(WeT_sb[:, 2:, :], WeTb[:])
            nc.gpsimd.dma_start(xT_sb[:, 2:, :], xTb[:])
            nc.sync.dma_start(WdT_sb[:, :2, :], WdTa[:])
            nc.sync.dma_start(WdT_sb[:, 2:, :], WdTb[:])
            nc.scalar.dma_start(dT_sb[:], dT[:])
            nc.sync.dma_start(cb_sb[:], cb[:])
            nc.sync.dma_start(ob_sb[:], ob[:])
            nc.gpsimd.dma_start(Wo_sb[:, :2, :], WoT[:, :2, :])
            nc.gpsimd.dma_start(Wo_sb[:, 2:, :], WoT[:, 2:, :])

            # ---- projections -------------------------------------------------
            # enc_T[J,T]: lhsT = WeT chunk [E_k, J_m], rhs = xT chunk [E_k, T]
            # ek outer so the first matmuls only need chunks 0-1 of the DMAs;
            # enc and dec interleaved per chunk so each DMA round feeds both.
            ps_enc = [
                psum_pool.tile([P, T], F32, tag="ps", name=f"ps_enc_{jm}")
                for jm in range(JC)
            ]
            ps_dec = [
                psum_pool.tile([P, T], F32, tag="ps", name=f"ps_dec_{jm}")
                for jm in range(JC)
            ]

            # PE warm-up: dummy matmuls while the input DMAs land, so the HAM
            # clock-gate lifts (1.2 -> 2.4 GHz) before the projections issue.
            # Target ps_dec[3]'s bank (the last projection group to issue) so
            # the WAW ordering with the warm-up delays the pipeline least.
            # The group's start=True resets the bank afterwards.  The count is
            # sized so the last dummy ends right as the WeT/xT half1 DMA
            # semaphores fire - any longer delays the projections, any shorter
            # leaves a PE idle gap that re-throttles the clock to 1.2 GHz.
            # dummy tanh while the input DMAs land: hoists ScalarE's 1.3 us
            # ACT_TABLE_LOAD off the dec_sb -> first-tanh critical path
            nc.scalar.activation(
                warm_act[:], warm_sb[:], mybir.ActivationFunctionType.Tanh
            )
            for w in range(WARM_A):
                nc.tensor.matmul(
                    ps_dec[3][:64, :64],
                    lhsT=warm_sb[:, :64],
                    rhs=warm_sb[:],
                    start=True,
                    stop=True,
                    skip_group_check=True,
                )

            # ek-outer: an ek pass touches only one 128-row chunk of WeT/xT,
            # so the first 8 enc matmuls gate only on the half1 DMAs (256 KB)
            # instead of the whole 512 KB tensors.  All four enc (and dec)
            # PSUM groups accumulate in parallel and close on the ek=3 pass.
            def enc_pass(ek):
                for jm in range(JC):
                    nc.tensor.matmul(
                        ps_enc[jm][:],
                        lhsT=WeT_sb[:, ek, ts(jm, P)],
                        rhs=xT_sb[:, ek, :],
                        start=(ek == 0),
                        stop=(ek == EC - 1),
                        skip_group_check=True,
                    )

            def dec_pass(ek):
                for jm in range(JC):
                    nc.tensor.matmul(
                        ps_dec[jm][:, :U],
                        lhsT=WdT_sb[:, ek, ts(jm, P)],
                        rhs=dT_sb[:, ek, :],
                        start=(ek == 0),
                        stop=(ek == EC - 1),
                        skip_group_check=True,
                    )

            def enc_mm(jm, ek, stop=False):
                nc.tensor.matmul(
                    ps_enc[jm][:],
                    lhsT=WeT_sb[:, ek, ts(jm, P)],
                    rhs=xT_sb[:, ek, :],
                    start=False,
                    stop=stop,
                    skip_group_check=True,
                )

            enc_pass(0)
            enc_pass(1)
            # jm=0 closes FIRST so its DVE evac (halved: tanh needs only
            # T-half0 to start) and the scalar chain behind it overlap the
            # rest of the projections.  dec sits after the jm1-3 ek=2 filler
            # because its WdT/dT semaphores are the last to fire; its ScalarE
            # bias-add evacuations then drain during the final enc matmuls.
            enc_mm(0, 2)
            enc_mm(0, 3, stop=True)
            nc.vector.tensor_copy(enc_sb[:, 0, : T // 2], ps_enc[0][:, : T // 2])
            nc.vector.tensor_copy(enc_sb[:, 0, T // 2 :], ps_enc[0][:, T // 2 :])
            for jm in (1, 2, 3):
                enc_mm(jm, 2)
            dec_pass(0)
            dec_pass(1)
            dec_pass(2)
            dec_pass(3)

            def dec_evac(jm):
                nc.scalar.activation(
                    dec_sb[:, jm, :],
                    ps_dec[jm][:, :U],
                    mybir.ActivationFunctionType.Identity,
                    bias=cb_sb[:, jm : jm + 1],
                )

            # only jm=0's bias-add is issued here; jm1-3 are interleaved into
            # the u=0 tanh loop so they don't sit ahead of tanh(u0, jc0) on
            # the in-order ScalarE queue (they'd delay the first u matmul)
            dec_evac(0)
            for jm in (1, 2, 3):
                enc_mm(jm, 3, stop=True)
                nc.vector.tensor_copy(enc_sb[:, jm, :], ps_enc[jm][:])

            # ---- main loop over u -------------------------------------------
            for u in range(U):
                act_t = act_pool.tile([P, JC, T], BF16, tag="act", name=f"act_{u}")
                for jc in range(JC):
                    if u == 0 and jc == 0:
                        for h in range(2):
                            nc.scalar.activation(
                                act_t[:, 0, ts(h, T // 2)],
                                enc_sb[:, 0, ts(h, T // 2)],
                                mybir.ActivationFunctionType.Tanh,
                                bias=dec_sb[:, 0, u : u + 1],
                            )
                    else:
                        if u == 0:
                            dec_evac(jc)
                        nc.scalar.activation(
                            act_t[:, jc, :],
                            enc_sb[:, jc, :],
                            mybir.ActivationFunctionType.Tanh,
                            bias=dec_sb[:, jc, u : u + 1],
                        )
                ot = out_pool.tile([P, TB, V], BF16, tag="ot", name=f"ot_{u}")
                # First two u: jc-outer over the four tb accumulation groups,
                # so the first four matmuls need only tanh chunk 0 and the PE
                # never stalls on a chunk ScalarE hasn't produced yet.
                # Later u (act always ready ahead): tb-outer, so each group
                # closes early and its evacuation overlaps the remaining
                # matmuls - keeps the final-u evacs off the tail.
                pss = [
                    psum_pool.tile([P, T], F32, tag="ps", name=f"ps_{u}_{tb}")
                    for tb in range(TB)
                ]
                order = (
                    [(jc, tb) for jc in range(JC) for tb in range(TB)]
                    if u < 2
                    else [(jc, tb) for tb in range(TB) for jc in range(JC)]
                )
                for jc, tb in order:
                    nc.tensor.matmul(
                        pss[tb][:, :V],
                        lhsT=act_t[:, jc, ts(tb, P)],
                        rhs=Wo_sb[:, jc, :],
                        start=(jc == 0),
                        stop=(jc == JC - 1),
                        skip_group_check=True,
                    )
                for tb in range(TB):
                    ps = pss[tb]
                    nc.vector.tensor_add(ot[:, tb, :], ps[:, :V], ob_sb[:])
                    if u == U - 1:
                        # final u: early tb-pair store, then single-tb stores
                        # so the last transfer (the tail's critical path) is
                        # only 128 KB and departs as soon as its evac lands.
                        if tb == 1:
                            nc.sync.dma_start(out[u, :, :2, :], ot[:, :2, :])
                        elif tb == 2:
                            nc.sync.dma_start(out[u, :, 2:3, :], ot[:, 2:3, :])
                        elif tb == TB - 1:
                            nc.gpsimd.dma_start(out[u, :, 3:, :], ot[:, 3:, :])
                if u < U - 1:
                    # one contiguous 512 KB store per u (4000 B per partition);
                    # per-tb slices of this layout have only 1000 B lines, so
                    # splitting penultimate stores measurably loses bandwidth
                    nc.sync.dma_start(out[u], ot[:])

    nc.compile()
    return nc


_CACHED_NC = None


def _get_nc():
    global _CACHED_NC
    if _CACHED_NC is None:
        _CACHED_NC = build_nc()
    return _CACHED_NC


def make_in_maps(
    encoder_out, decoder_out, enc_W, enc_b, dec_W, dec_b, out_W, out_b
) -> list[dict]:
    bf = ml_dtypes.bfloat16
    f32 = np.float32

    def t_pc(a):  # [F, K] -> K on partitions chunk-major: [P, K//P, F], bf16
        a = np.asarray(a, dtype=f32).T  # [K, F]
        k, f = a.shape
        return np.ascontiguousarray(
            a.reshape(k // P, P, f).transpose(1, 0, 2)
        ).astype(bf)

    WeT = t_pc(enc_W)  # [P, EC, J]
    WdT = t_pc(dec_W)  # [P, EC, J]
    WoT = t_pc(out_W)  # [P, JC, V]
    cb = (
        (np.asarray(enc_b, f32) + np.asarray(dec_b, f32))
        .reshape(JC, P)
        .T.copy()
    )  # [P, JC]
    ob = np.broadcast_to(
        np.asarray(out_b, f32).astype(bf).reshape(1, V), (P, V)
    ).copy()  # [P, V] replicated

    encoder_out = np.asarray(encoder_out, f32)
    decoder_out = np.asarray(decoder_out, f32)

    def halves(a):  # [P, EC, F] -> two contiguous [P, 2, F]
        return (
            np.ascontiguousarray(a[:, :2, :]),
            np.ascontiguousarray(a[:, 2:, :]),
        )

    WeTa, WeTb = halves(WeT)
    WdTa, WdTb = halves(WdT)
    in_maps = []
    for i in range(NUM_CORES):
        xTa, xTb = halves(t_pc(encoder_out[i]))
        in_maps.append(
            {
                "xTa": xTa,
                "xTb": xTb,
                "dT": t_pc(decoder_out[i]),  # [P, EC, U]
                "WeTa": WeTa,
                "WeTb": WeTb,
                "WdTa": WdTa,
                "WdTb": WdTb,
                "WoT": WoT,
                "cb": cb,
                "ob": ob,
            }
        )
    return in_maps


def unscramble(dev_out: np.ndarray) -> np.ndarray:
    """[U, P, TB, V] bf16 device layout -> [T, U, V] fp32 (t = tb*P + p)."""
    return (
        np.asarray(dev_out)
        .transpose(2, 1, 0, 3)  # [TB, P, U, V]
        .reshape(T, U, V)
        .astype(np.float32)
    )


def run(inputs: dict, trace: bool = False):
    """Returns (full_output, BassKernelResults)."""
    nc = _get_nc()
    in_maps = make_in_maps(**inputs)
    res = run_bass_kernel_spmd(
        nc, in_maps, core_ids=list(range(NUM_CORES)), trace=trace
    )
    out = np.stack(
        [unscramble(res.results[i]["out"]) for i in range(NUM_CORES)], axis=0
    )  # (N, T, U, V)
    return np.ascontiguousarray(out, dtype=np.float32), res


def kernel(**inputs) -> np.ndarray:
    out, _ = run(inputs, trace=False)
    return out

